# revision 1
# baseline (speedup 1.0000x reference)
"""Trainium2 Bass kernel for nn_ContextAttention (sparse_attention).

Math (per batch b):
  q = (x @ Wq + bq) / 16 ; k = x @ Wk + bk ; v0 = x @ Wv   (bv folded into bout)
  scoresT[t,s] = sum_d kT[d,t] qT[d,s] + pe[t,s]           (pe symmetric)
  E1 = exp(scoresT), E2 = E1 * band(|s-t|<=32)             (maskless softmax, scores are small)
  O1uT[d,s] = sum_t V~[t,d] E1[t,s]  with V~=[V|1] -> row 64 = denominator d1[s]
  OT = O1uT/d1 + O2uT/d2   (x0.5 folded into Wout)
  out = OT.T @ (0.5*Wout) + (bv @ Wout + bout)

Sharding: data-parallel over batch across 8 cores (8 batches each). No collectives.
"""

import sys

sys.path.insert(0, "/opt/trn_rl_repo")

import numpy as np

B, S, F, E, H, DH = 64, 512, 512, 256, 4, 64
HALF_WIN = 32
SCALE = 16.0  # EMBED ** 0.5
NCORES = 8
BPC = B // NCORES  # batches per core
TOK = BPC * S  # tokens per core
F32 = None  # set after imports


def _build():
    import concourse.bacc as bacc
    import concourse.tile as tile
    from concourse import mybir

    f32 = mybir.dt.float32
    f32r = mybir.dt.float32r
    Copy = mybir.ActivationFunctionType.Copy
    Exp = mybir.ActivationFunctionType.Exp
    mult = mybir.AluOpType.mult
    add = mybir.AluOpType.add

    nc = bacc.Bacc("TRN2", target_bir_lowering=False, debug=False)

    xT = nc.dram_tensor("xT", [F, TOK], f32r, kind="ExternalInput")
    wq_d = nc.dram_tensor("wq", [F, E], f32r, kind="ExternalInput")
    wk_d = nc.dram_tensor("wk", [F, E], f32r, kind="ExternalInput")
    wv_d = nc.dram_tensor("wv", [F, E], f32r, kind="ExternalInput")
    wout_d = nc.dram_tensor("wout", [E, F], f32r, kind="ExternalInput")
    qkb_d = nc.dram_tensor("qkbias", [128, 4], f32, kind="ExternalInput")
    bout_d = nc.dram_tensor("boutr", [1, F], f32, kind="ExternalInput")
    pet_d = nc.dram_tensor("pet", [S, S], f32r, kind="ExternalInput")
    mt_d = nc.dram_tensor("mt", [S, S], f32r, kind="ExternalInput")
    id_d = nc.dram_tensor("ident", [128, 128], f32r, kind="ExternalInput")
    out_d = nc.dram_tensor("out", [TOK, F], f32, kind="ExternalOutput")

    def r(ap):
        return ap

    with tile.TileContext(nc) as tc:
        with (
            tc.tile_pool(name="const", bufs=1) as const,
            tc.tile_pool(name="xt", bufs=2) as xpool,
            tc.tile_pool(name="qk", bufs=2) as qkpool,
            tc.tile_pool(name="vt", bufs=2) as vpool,
            tc.tile_pool(name="ee", bufs=8) as epool,
            tc.tile_pool(name="rr", bufs=4) as rpool,
            tc.tile_pool(name="rb", bufs=4) as rbpool,
            tc.tile_pool(name="nn", bufs=2) as npool,
            tc.tile_pool(name="ot", bufs=2) as otpool,
            tc.tile_pool(name="ff", bufs=3) as fpool,
            tc.tile_pool(name="ps", bufs=4, space="PSUM") as pspool,
            tc.tile_pool(name="pso", bufs=4, space="PSUM") as psopool,
        ):
            # ---- persistent constants ----
            wq_sb, wk_sb, wv_sb = [], [], []
            for kc in range(4):
                for wn, lst, dram in (("wq", wq_sb, wq_d), ("wk", wk_sb, wk_d), ("wv", wv_sb, wv_d)):
                    t = const.tile([128, E], f32r, name=f"{wn}_{kc}", tag=f"{wn}{kc}")
                    nc.sync.dma_start(t[:], dram[128 * kc : 128 * (kc + 1), :])
                    lst.append(t)
            wout_sb = []
            for c in range(2):
                t = const.tile([128, F], f32r, tag=f"wout{c}")
                nc.sync.dma_start(t[:], wout_d[128 * c : 128 * (c + 1), :])
                wout_sb.append(t)
            pet_sb, mt_sb = [], []
            for tt in range(4):
                t = const.tile([128, S], f32r, tag=f"pet{tt}")
                nc.sync.dma_start(t[:], pet_d[128 * tt : 128 * (tt + 1), :])
                pet_sb.append(t)
                t = const.tile([128, S], f32r, tag=f"mt{tt}")
                nc.sync.dma_start(t[:], mt_d[128 * tt : 128 * (tt + 1), :])
                mt_sb.append(t)
            id_sb = const.tile([128, 128], f32r, tag="ident")
            nc.sync.dma_start(id_sb[:], id_d[:, :])
            qkb_sb = const.tile([128, 4], f32, tag="qkb")
            nc.sync.dma_start(qkb_sb[:], qkb_d[:, :])
            bout_row = const.tile([1, F], f32, tag="boutrow")
            nc.sync.dma_start(bout_row[:], bout_d[0:1, :])
            bout_b = const.tile([128, F], f32, tag="boutb")
            nc.gpsimd.partition_broadcast(bout_b[:], bout_row[:])

            for b in range(BPC):
                # ---- load xT slice for this batch ----
                xt = []
                for kc in range(4):
                    t = xpool.tile([128, S], f32r, tag=f"xt{kc}")
                    nc.sync.dma_start(
                        t[:], xT[128 * kc : 128 * (kc + 1), 512 * b : 512 * (b + 1)]
                    )
                    xt.append(t)

                # ---- Q^T / K^T projections (e on partitions) ----
                QP, KP = [], []
                for et in range(2):
                    for lst, w_sb, bcol in ((QP, wq_sb, 0), (KP, wk_sb, 2)):
                        ps = pspool.tile([128, S], f32, tag="ps")
                        for kc in range(4):
                            nc.tensor.matmul(
                                ps[:],
                                r(w_sb[kc][:, 128 * et : 128 * (et + 1)]),
                                r(xt[kc][:]),
                                start=(kc == 0),
                                stop=(kc == 3),
                            )
                        t = qkpool.tile([128, S], f32r, tag=f"{'q' if bcol == 0 else 'k'}p{et}")
                        nc.scalar.add(t[:], ps[:], qkb_sb[:, bcol + et : bcol + et + 1])
                        lst.append(t)

                # ---- V projection ([t, e] layout) + ones columns ----
                Vt = []
                for j in range(4):
                    ps = pspool.tile([128, E], f32, tag="ps")
                    for kc in range(4):
                        nc.tensor.matmul(
                            ps[:],
                            r(xt[kc][:, 128 * j : 128 * (j + 1)]),
                            r(wv_sb[kc][:]),
                            start=(kc == 0),
                            stop=(kc == 3),
                        )
                    vt = vpool.tile([128, 4 * 65], f32r, tag=f"vt{j}")
                    nc.scalar.activation(
                        vt.rearrange("p (h x) -> p h x", x=65)[:, :, 0:64],
                        ps.rearrange("p (h x) -> p h x", x=64),
                        Copy,
                    )
                    nc.vector.memset(
                        vt.bitcast(f32).rearrange("p (h x) -> p h x", x=65)[:, :, 64:65],
                        1.0,
                    )
                    Vt.append(vt)

                # ---- attention per head ----
                OT = [
                    otpool.tile([128, S], f32r, name=f"ot{c}_{b}", tag=f"ot{c}")
                    for c in range(2)
                ]
                for h in range(H):
                    et, hl = h // 2, h % 2
                    E1s, E2s = [], []
                    for tt in range(4):
                        sp = pspool.tile([128, S], f32, tag="ps")
                        nc.tensor.matmul(
                            sp[:],
                            r(KP[et][64 * hl : 64 * hl + 64, 128 * tt : 128 * (tt + 1)]),
                            r(QP[et][64 * hl : 64 * hl + 64, :]),
                            start=True,
                            stop=False,
                        )
                        nc.tensor.matmul(
                            sp[:], r(id_sb[:]), r(pet_sb[tt][:]), start=False, stop=True
                        )
                        e1 = epool.tile([128, S], f32r, tag="e1")
                        nc.scalar.activation(e1[:], sp[:], Exp)
                        e2 = epool.tile([128, S], f32r, tag="e2")
                        nc.vector.tensor_tensor(e2[:], e1[:], mt_sb[tt][:], mult)
                        E1s.append(e1)
                        E2s.append(e2)
                    o1 = psopool.tile([65, S], f32, tag="po")
                    o2 = psopool.tile([65, S], f32, tag="po")
                    for tt in range(4):
                        nc.tensor.matmul(
                            o1[:],
                            r(Vt[tt][:, 65 * h : 65 * h + 65]),
                            r(E1s[tt][:]),
                            start=(tt == 0),
                            stop=(tt == 3),
                        )
                    for tt in range(4):
                        nc.tensor.matmul(
                            o2[:],
                            r(Vt[tt][:, 65 * h : 65 * h + 65]),
                            r(E2s[tt][:]),
                            start=(tt == 0),
                            stop=(tt == 3),
                        )
                    rt = rpool.tile([65, S], f32, tag="rt")
                    nc.vector.reciprocal(rt[64:65, 0:S], o1[64:65, :])
                    rs1 = rpool.tile([1, S], f32, tag="rs1")
                    nc.sync.dma_start(rs1[:], rt[64:65, 0:S])
                    rb1 = rbpool.tile([128, S], f32, tag="rb")
                    nc.gpsimd.partition_broadcast(rb1[:], rs1[0:1, 0:S])
                    rt2 = rpool.tile([65, S], f32, tag="rt")
                    nc.vector.reciprocal(rt2[64:65, 0:S], o2[64:65, :])
                    rs2 = rpool.tile([1, S], f32, tag="rs2")
                    nc.sync.dma_start(rs2[:], rt2[64:65, 0:S])
                    rb2 = rbpool.tile([128, S], f32, tag="rb")
                    nc.gpsimd.partition_broadcast(rb2[:], rs2[0:1, 0:S])
                    t1 = npool.tile([64, S], f32, tag="t1")
                    nc.vector.tensor_tensor(t1[:], o1[0:64, :], rb1[0:64, :], mult)
                    t2 = npool.tile([64, S], f32, tag="t2")
                    nc.vector.tensor_tensor(t2[:], o2[0:64, :], rb2[0:64, :], mult)
                    if hl == 0:
                        nc.vector.tensor_tensor(
                            OT[et][0:64, :], t1[:], t2[:], add
                        )
                    else:
                        tmp = npool.tile([64, S], f32r, tag="tmp")
                        nc.vector.tensor_tensor(tmp[:], t1[:], t2[:], add)
                        nc.sync.dma_start(OT[et][64:128, :], tmp[:])

                # ---- output projection ----
                for j in range(4):
                    fp = pspool.tile([128, F], f32, tag="ps")
                    nc.tensor.matmul(
                        fp[:],
                        r(OT[0][:, 128 * j : 128 * (j + 1)]),
                        r(wout_sb[0][:]),
                        start=True,
                        stop=False,
                    )
                    nc.tensor.matmul(
                        fp[:],
                        r(OT[1][:, 128 * j : 128 * (j + 1)]),
                        r(wout_sb[1][:]),
                        start=False,
                        stop=True,
                    )
                    fs = fpool.tile([128, F], f32, tag="fs")
                    nc.vector.tensor_tensor(fs[:], fp[:], bout_b[:], add)
                    row = 512 * b + 128 * j
                    nc.sync.dma_start(out_d[row : row + 128, :], fs[:])

    nc.compile()
    return nc


_CACHE = {}
LAST_RESULTS = None


def prep_in_maps(inputs, Wq, bq, Wk, bk, Wv, bv, gamma, theta, Wout, bout):
    x = np.asarray(inputs, np.float32)
    Wq = np.asarray(Wq, np.float32)
    bq = np.asarray(bq, np.float32)
    Wk = np.asarray(Wk, np.float32)
    bk = np.asarray(bk, np.float32)
    Wv = np.asarray(Wv, np.float32)
    bv = np.asarray(bv, np.float32)
    Wout = np.asarray(Wout, np.float32)
    bout = np.asarray(bout, np.float32)
    gamma = float(np.asarray(gamma))
    theta = float(np.asarray(theta))

    # host-side prep
    wq_s = Wq / SCALE
    bq_s = bq / SCALE
    idx = np.arange(S)
    diff = (idx[:, None] - idx[None, :]).astype(np.float32)
    pe = np.exp(-np.abs(gamma * diff * diff - theta)).astype(np.float32)  # symmetric
    band = (np.abs(diff) <= HALF_WIN).astype(np.float32)  # symmetric
    qkb = np.stack(
        [bq_s[:128], bq_s[128:], bk[:128], bk[128:]], axis=1
    ).astype(np.float32)  # [128, 4]
    bout_p = (bout + bv @ Wout).astype(np.float32).reshape(1, F)
    wout_h = (0.5 * Wout).astype(np.float32)
    ident = np.eye(128, dtype=np.float32)

    shared = {
        "wq": np.ascontiguousarray(wq_s),
        "wk": np.ascontiguousarray(Wk),
        "wv": np.ascontiguousarray(Wv),
        "wout": np.ascontiguousarray(wout_h),
        "qkbias": np.ascontiguousarray(qkb),
        "boutr": bout_p,
        "pet": np.ascontiguousarray(pe),
        "mt": np.ascontiguousarray(band),
        "ident": ident,
    }
    in_maps = []
    for c in range(NCORES):
        xc = x[c * BPC : (c + 1) * BPC].reshape(TOK, F)
        m = dict(shared)
        m["xT"] = np.ascontiguousarray(xc.T)
        in_maps.append(m)
    return in_maps


def get_nc():
    if "nc" not in _CACHE:
        _CACHE["nc"] = _build()
    return _CACHE["nc"]


def kernel(inputs, Wq, bq, Wk, bk, Wv, bv, gamma, theta, Wout, bout):
    global LAST_RESULTS
    from concourse.bass_utils import run_bass_kernel_spmd

    in_maps = prep_in_maps(
        inputs, Wq, bq, Wk, bk, Wv, bv, gamma, theta, Wout, bout
    )
    nc = get_nc()
    res = run_bass_kernel_spmd(nc, in_maps, core_ids=list(range(NCORES)))
    LAST_RESULTS = res
    out = np.concatenate(
        [res.results[c]["out"].reshape(BPC, S, F) for c in range(NCORES)], axis=0
    )
    return out

